# revision 1
# baseline (speedup 1.0000x reference)
"""Multi-head graph attention message passing on 8 Trainium2 cores.

Strategy (graph/data parallel, per the dst-sharding scheme):
  - Nodes sharded by dst across 8 cores (12500 each). Each core owns the
    wV rows for its dst range; segment_sum is local via hardware
    dma_scatter_add (CCE accumulate in the SDMA datapath).
  - Q/K/V projections: small weights replicated; every core computes the
    full K,V tables (replicated compute, no cross-core traffic) and the
    Q table for its own node range only. K,V stored interleaved per node
    row [K|V] so one dma_gather per edge fetches both.
  - Edges routed by dst partition on host; within a core, grouped by src
    chunk (4 chunks) so dma_gather int16 indices stay in range.
"""

import numpy as np

import concourse.bacc as bacc
import concourse.mybir as mybir
import concourse.tile as tile
from concourse.bass_utils import run_bass_kernel_spmd

F32 = mybir.dt.float32
I16 = mybir.dt.int16


class Cfg:
    n_nodes = 100000
    n_edges = 1600000
    in_dim = 128
    heads = 8
    hdim = 16
    hid = 128          # heads * hdim
    n_cores = 8
    n_chunks = 4       # src chunks for int16 gather indices
    batch = 1024       # edges per device batch (dma_gather caps near 1024 idxs/call)
    proj_tile = 512    # nodes per projection DMA group
    kv_bf16 = False    # store K,V tables in bf16 (halves gather traffic)

    def __init__(self, **kw):
        for k, v in kw.items():
            setattr(self, k, v)
        assert self.n_nodes % self.n_cores == 0
        self.own = self.n_nodes // self.n_cores
        # padded full node count: multiple of proj_tile and n_chunks
        m = self.proj_tile * self.n_chunks
        self.np_pad = -(-self.n_nodes // m) * m
        self.cr = self.np_pad // self.n_chunks          # chunk rows
        assert self.cr <= 32767, "gather idx must fit int16"
        self.own_pad = -(-self.own // self.proj_tile) * self.proj_tile
        self.wv_rows = self.own_pad + 128  # spare rows for the padding sink
        assert self.wv_rows <= 32767
        self.dummy_row = self.own_pad  # scatter target for padding edges


def build_program(cfg, g_pad):
    """One SPMD program; per-core behavior differs only through input data."""
    nc = bacc.Bacc("TRN2", target_bir_lowering=False, debug=False)
    W = g_pad // 16

    xt = nc.dram_tensor("xt", [cfg.in_dim, cfg.np_pad], F32, kind="ExternalInput")
    xt_own = nc.dram_tensor("xt_own", [cfg.in_dim, cfg.own_pad], F32, kind="ExternalInput")
    w_kv = nc.dram_tensor("w_kv", [cfg.in_dim, 2 * cfg.hid], F32, kind="ExternalInput")
    w_q = nc.dram_tensor("w_q", [cfg.in_dim, cfg.hid], F32, kind="ExternalInput")
    b_kv = nc.dram_tensor("b_kv", [128, 2 * cfg.hid], F32, kind="ExternalInput")
    b_q = nc.dram_tensor("b_q", [128, cfg.hid], F32, kind="ExternalInput")
    kv_idx = nc.dram_tensor("kv_idx", [cfg.n_chunks, 128, W], I16, kind="ExternalInput")
    q_idx = nc.dram_tensor("q_idx", [cfg.n_chunks, 128, W], I16, kind="ExternalInput")
    sc_idx = nc.dram_tensor("sc_idx", [cfg.n_chunks, 128, W], I16, kind="ExternalInput")

    wv = nc.dram_tensor("wv", [cfg.wv_rows, cfg.hid], F32, kind="ExternalOutput")

    KVDT = mybir.dt.bfloat16 if cfg.kv_bf16 else F32
    kv_tab = nc.dram_tensor("kv_tab", [cfg.np_pad, 2 * cfg.hid], KVDT)
    q_tab = nc.dram_tensor("q_tab", [cfg.own_pad, cfg.hid], F32)

    PT = cfg.proj_tile
    B = cfg.batch
    BC = B // 128  # column groups per batch tile

    with tile.TileContext(nc) as tc:
        with (
            tc.tile_pool(name="const", bufs=1) as cpool,
            tc.tile_pool(name="proj", bufs=3) as ppool,
            tc.tile_pool(name="psum", bufs=4, space="PSUM") as psum,
            tc.tile_pool(name="edge", bufs=3) as epool,
            tc.tile_pool(name="idx", bufs=3) as ipool,
        ):
            w_kv_t = cpool.tile([cfg.in_dim, 2 * cfg.hid], F32)
            w_q_t = cpool.tile([cfg.in_dim, cfg.hid], F32)
            b_kv_t = cpool.tile([128, 2 * cfg.hid], F32)
            b_q_t = cpool.tile([128, cfg.hid], F32)
            nc.sync.dma_start(w_kv_t[:], w_kv[:])
            nc.sync.dma_start(w_q_t[:], w_q[:])
            nc.sync.dma_start(b_kv_t[:], b_kv[:])
            nc.sync.dma_start(b_q_t[:], b_q[:])

            zt = cpool.tile([128, 4 * cfg.hid], F32)
            nc.vector.memset(zt[:], 0.0)
            for r in range(0, cfg.wv_rows, 512):
                rows = min(512, cfg.wv_rows - r)
                zview = wv[r:r + rows, :].rearrange("(s p) e -> p s e", p=128)
                nc.sync.dma_start(
                    zview, zt[:, :rows].rearrange("p (s e) -> p s e", e=cfg.hid))

            def project(src_dram, n_pad, w_t, b_t, out_dram, out_width, odt=F32):
                for g in range(n_pad // PT):
                    xt_t = ppool.tile([128, PT], F32, tag="xt_t")
                    nc.sync.dma_start(xt_t[:], src_dram[:, g * PT:(g + 1) * PT])
                    out_sb = ppool.tile([128, PT // 128, out_width], odt, tag="out_sb")
                    for s in range(PT // 128):
                        ps = psum.tile([128, out_width], F32)
                        nc.tensor.matmul(
                            ps[:], xt_t[:, s * 128:(s + 1) * 128], w_t[:],
                            start=True, stop=True,
                        )
                        nc.vector.tensor_add(out_sb[:, s, :], ps[:], b_t[:])
                    dview = out_dram[g * PT:(g + 1) * PT, :].rearrange(
                        "(s p) e -> p s e", p=128)
                    nc.sync.dma_start(dview, out_sb[:])

            project(xt, cfg.np_pad, w_kv_t, b_kv_t, kv_tab, 2 * cfg.hid, KVDT)
            project(xt_own, cfg.own_pad, w_q_t, b_q_t, q_tab, cfg.hid)

            for ch in range(cfg.n_chunks):
                kv_src = kv_tab[ch * cfg.cr:(ch + 1) * cfg.cr, :]
                for b in range(g_pad // B):
                    c0 = b * (B // 16)
                    kvi = ipool.tile([128, B // 16], I16, tag="kvi")
                    qi = ipool.tile([128, B // 16], I16, tag="qi")
                    sci = ipool.tile([128, B // 16], I16, tag="sci")
                    nc.sync.dma_start(kvi[:], kv_idx[ch, :, c0:c0 + B // 16])
                    nc.sync.dma_start(qi[:], q_idx[ch, :, c0:c0 + B // 16])
                    nc.sync.dma_start(sci[:], sc_idx[ch, :, c0:c0 + B // 16])

                    kv_t = epool.tile([128, BC, 2 * cfg.hid], KVDT, tag="kv_t")
                    q_t = epool.tile([128, BC, cfg.hid], F32, tag="q_t")
                    nc.gpsimd.dma_gather(
                        kv_t[:], kv_src, kvi[:], B, B, 2 * cfg.hid)
                    nc.gpsimd.dma_gather(
                        q_t[:], q_tab[:], qi[:], B, B, cfg.hid)

                    prod = epool.tile([128, BC, cfg.hid], F32, tag="prod")
                    nc.vector.tensor_mul(prod[:], kv_t[:, :, :cfg.hid], q_t[:])

                    sc = epool.tile([128, BC, cfg.heads], F32, tag="sc")
                    nc.vector.reduce_sum(
                        sc[:],
                        prod[:].rearrange("p c (h d) -> p c h d", d=cfg.hdim),
                        axis=mybir.AxisListType.X,
                    )
                    # clip(dot/scale, ±5) == clip(dot, ±5*scale) then /scale
                    lim = 5.0 * float(np.sqrt(cfg.hdim))
                    nc.vector.tensor_scalar_min(sc[:], sc[:], lim)
                    nc.vector.tensor_scalar_max(sc[:], sc[:], -lim)
                    ex = epool.tile([128, BC, cfg.heads], F32, tag="ex")
                    nc.scalar.activation(
                        ex[:], sc[:], mybir.ActivationFunctionType.Exp,
                        scale=float(1.0 / np.sqrt(cfg.hdim)),
                    )

                    msg = epool.tile([128, BC, cfg.hid], F32, tag="msg")
                    nc.vector.tensor_mul(
                        msg[:].rearrange("p c (h d) -> p c h d", d=cfg.hdim),
                        kv_t[:, :, cfg.hid:].rearrange(
                            "p c (h d) -> p c h d", d=cfg.hdim),
                        ex[:].unsqueeze(-1).broadcast_to(
                            [128, BC, cfg.heads, cfg.hdim]),
                    )
                    nc.gpsimd.dma_scatter_add(
                        wv[:], msg[:], sci[:], B, B, cfg.hid)
    nc.finalize()
    return nc


def _wrap16(a, g_pad):
    """[n] -> [128, g_pad//16] int16: idx i at [i%16 (+16k replicas), i//16]."""
    w = a.reshape(g_pad // 16, 16).T.astype(np.int16)  # [16, W]
    return np.tile(w, (8, 1))


def _schedule_batches(dst_local, batch):
    """Assign edges to batches of size `batch` so that no dst row appears
    twice within one batch (dma_scatter_add RMW races on duplicate rows
    within a single call). Returns (n_batches, edge order as an index
    array grouped by batch, per-batch counts)."""
    cnt = len(dst_local)
    if cnt == 0:
        return 1, np.empty(0, np.int64), np.zeros(1, np.int64)
    order = np.argsort(dst_local, kind="stable")
    uniq, starts, degs = np.unique(
        dst_local[order], return_index=True, return_counts=True)
    nb = max(-(-cnt // batch), int(degs.max()))
    big_first = np.argsort(-degs, kind="stable")
    while True:
        fills = np.zeros(nb, np.int64)
        bin_of = np.empty(cnt, np.int64)
        ok = True
        for gi in big_first:
            d = degs[gi]
            cand = np.argsort(fills, kind="stable")[:d]
            if fills[cand[-1]] >= batch:
                ok = False
                break
            fills[cand] += 1
            s = starts[gi]
            bin_of[order[s:s + d]] = cand
        if ok:
            break
        nb += 1
    batch_order = np.argsort(bin_of, kind="stable")
    counts = np.bincount(bin_of, minlength=nb)
    return nb, batch_order, counts


def prepare_inputs(cfg, x, src, dst, Wq, bq, Wk, bk, Wv, bv):
    x = np.asarray(x, np.float32)
    src = np.asarray(src, np.int64)
    dst = np.asarray(dst, np.int64)

    xt = np.zeros((cfg.in_dim, cfg.np_pad), np.float32)
    xt[:, :cfg.n_nodes] = x.T
    w_kv = np.concatenate([np.asarray(Wk, np.float32),
                           np.asarray(Wv, np.float32)], axis=1)
    b_kv = np.tile(np.concatenate([np.asarray(bk, np.float32),
                                   np.asarray(bv, np.float32)])[None, :], (128, 1))
    w_q = np.asarray(Wq, np.float32)
    b_q = np.tile(np.asarray(bq, np.float32)[None, :], (128, 1))

    core_of = dst // cfg.own
    chunk_of = src // cfg.cr

    # per-(core, chunk) edge lists, scheduled into duplicate-free batches
    groups = {}
    nb_max = 1
    for c in range(cfg.n_cores):
        in_c = np.nonzero(core_of == c)[0]
        ch_c = chunk_of[in_c]
        for ch in range(cfg.n_chunks):
            e = in_c[ch_c == ch]
            nb, border, counts = _schedule_batches(
                (dst[e] - c * cfg.own), cfg.batch)
            groups[(c, ch)] = (e[border] if len(e) else e, counts)
            nb_max = max(nb_max, nb)
    g_pad = nb_max * cfg.batch

    in_maps = []
    for c in range(cfg.n_cores):
        kvi = np.zeros((cfg.n_chunks, 128, g_pad // 16), np.int16)
        qi = np.zeros((cfg.n_chunks, 128, g_pad // 16), np.int16)
        sci = np.zeros((cfg.n_chunks, 128, g_pad // 16), np.int16)
        for ch in range(cfg.n_chunks):
            e, counts = groups[(c, ch)]
            kv_l = np.zeros(g_pad, np.int64)
            q_l = np.zeros(g_pad, np.int64)
            sc_l = np.full(g_pad, cfg.dummy_row, np.int64)
            pos = 0
            off = 0
            for b, cnt in enumerate(counts):
                eb = e[pos:pos + cnt]
                kv_l[off:off + cnt] = src[eb] - ch * cfg.cr
                q_l[off:off + cnt] = dst[eb] - c * cfg.own
                sc_l[off:off + cnt] = dst[eb] - c * cfg.own
                pos += cnt
                off += cfg.batch
            kvi[ch] = _wrap16(kv_l, g_pad)
            qi[ch] = _wrap16(q_l, g_pad)
            sci[ch] = _wrap16(sc_l, g_pad)

        xt_own = np.zeros((cfg.in_dim, cfg.own_pad), np.float32)
        xt_own[:, :cfg.own] = x[c * cfg.own:(c + 1) * cfg.own].T
        in_maps.append({
            "xt": xt, "xt_own": xt_own,
            "w_kv": w_kv, "w_q": w_q, "b_kv": b_kv, "b_q": b_q,
            "kv_idx": kvi, "q_idx": qi, "sc_idx": sci,
        })
    return in_maps, g_pad


def kernel(x, src, dst, Wq, bq, Wk, bk, Wv, bv):
    cfg = Cfg()
    in_maps, g_pad = prepare_inputs(cfg, x, src, dst, Wq, bq, Wk, bk, Wv, bv)
    nc = build_program(cfg, g_pad)
    res = run_bass_kernel_spmd(nc, in_maps, list(range(cfg.n_cores)))
    out = np.concatenate(
        [res.results[c]["wv"][:cfg.own] for c in range(cfg.n_cores)], axis=0)
    return out.reshape(cfg.n_nodes, cfg.heads, cfg.hdim)



# revision 3
# speedup vs baseline: 1.4843x; 1.4843x over previous
"""Multi-head graph attention message passing on 8 Trainium2 cores.

Strategy (graph/data parallel, per the dst-sharding scheme):
  - Nodes sharded by dst across 8 cores (12500 each). Each core owns the
    wV rows for its dst range; segment_sum is local via hardware
    dma_scatter_add (CCE accumulate in the SDMA datapath).
  - Q/K/V projections: small weights replicated; every core computes the
    full K,V tables (replicated compute, no cross-core traffic) and the
    Q table for its own node range only. K,V stored interleaved per node
    row [K|V] so one dma_gather per edge fetches both.
  - Edges routed by dst partition on host; within a core, grouped by src
    chunk (4 chunks) so dma_gather int16 indices stay in range.
"""

import numpy as np

import concourse.bacc as bacc
import concourse.mybir as mybir
import concourse.tile as tile
from concourse.bass_utils import run_bass_kernel_spmd

F32 = mybir.dt.float32
I16 = mybir.dt.int16


class Cfg:
    n_nodes = 100000
    n_edges = 1600000
    in_dim = 128
    heads = 8
    hdim = 16
    hid = 128          # heads * hdim
    n_cores = 8
    n_chunks = 4       # src chunks for int16 gather indices
    batch = 1024       # edges per device batch (dma_gather caps near 1024 idxs/call)
    proj_tile = 512    # nodes per projection DMA group
    kv_bf16 = False    # store K,V tables in bf16 (halves gather traffic)

    def __init__(self, **kw):
        for k, v in kw.items():
            setattr(self, k, v)
        assert self.n_nodes % self.n_cores == 0
        self.own = self.n_nodes // self.n_cores
        # padded full node count: multiple of proj_tile and n_chunks
        m = self.proj_tile * self.n_chunks
        self.np_pad = -(-self.n_nodes // m) * m
        self.cr = self.np_pad // self.n_chunks          # chunk rows
        assert self.cr <= 32767, "gather idx must fit int16"
        self.own_pad = -(-self.own // self.proj_tile) * self.proj_tile
        self.wv_rows = self.own_pad + 128  # spare rows for the padding sink
        assert self.wv_rows <= 32767
        self.dummy_row = self.own_pad  # scatter target for padding edges


def build_program(cfg, g_pad):
    """One SPMD program; per-core behavior differs only through input data."""
    nc = bacc.Bacc("TRN2", target_bir_lowering=False, debug=False,
                   num_swdge_queues=4)
    W = g_pad // 16

    xt = nc.dram_tensor("xt", [cfg.in_dim, cfg.np_pad], F32, kind="ExternalInput")
    xt_own = nc.dram_tensor("xt_own", [cfg.in_dim, cfg.own_pad], F32, kind="ExternalInput")
    w_kv = nc.dram_tensor("w_kv", [cfg.in_dim, 2 * cfg.hid], F32, kind="ExternalInput")
    w_q = nc.dram_tensor("w_q", [cfg.in_dim, cfg.hid], F32, kind="ExternalInput")
    b_kv = nc.dram_tensor("b_kv", [128, 2 * cfg.hid], F32, kind="ExternalInput")
    b_q = nc.dram_tensor("b_q", [128, cfg.hid], F32, kind="ExternalInput")
    kv_idx = nc.dram_tensor("kv_idx", [cfg.n_chunks, 128, W], I16, kind="ExternalInput")
    q_idx = nc.dram_tensor("q_idx", [cfg.n_chunks, 128, W], I16, kind="ExternalInput")
    sc_idx = nc.dram_tensor("sc_idx", [cfg.n_chunks, 128, W], I16, kind="ExternalInput")

    wv = nc.dram_tensor("wv", [cfg.wv_rows, cfg.hid], F32, kind="ExternalOutput")

    KVDT = mybir.dt.bfloat16 if cfg.kv_bf16 else F32
    kv_tab = nc.dram_tensor("kv_tab", [cfg.np_pad, 2 * cfg.hid], KVDT)
    q_tab = nc.dram_tensor("q_tab", [cfg.own_pad, cfg.hid], F32)

    PT = cfg.proj_tile
    B = cfg.batch
    BC = B // 128  # column groups per batch tile

    with tile.TileContext(nc) as tc:
        with (
            tc.tile_pool(name="const", bufs=1) as cpool,
            tc.tile_pool(name="proj", bufs=3) as ppool,
            tc.tile_pool(name="psum", bufs=4, space="PSUM") as psum,
            tc.tile_pool(name="edge", bufs=3) as epool,
            tc.tile_pool(name="idx", bufs=3) as ipool,
        ):
            w_kv_t = cpool.tile([cfg.in_dim, 2 * cfg.hid], F32)
            w_q_t = cpool.tile([cfg.in_dim, cfg.hid], F32)
            b_kv_t = cpool.tile([128, 2 * cfg.hid], F32)
            b_q_t = cpool.tile([128, cfg.hid], F32)
            nc.sync.dma_start(w_kv_t[:], w_kv[:])
            nc.sync.dma_start(w_q_t[:], w_q[:])
            nc.sync.dma_start(b_kv_t[:], b_kv[:])
            nc.sync.dma_start(b_q_t[:], b_q[:])

            zt = cpool.tile([128, 4 * cfg.hid], F32)
            nc.vector.memset(zt[:], 0.0)
            for r in range(0, cfg.wv_rows, 512):
                rows = min(512, cfg.wv_rows - r)
                zview = wv[r:r + rows, :].rearrange("(s p) e -> p s e", p=128)
                nc.sync.dma_start(
                    zview, zt[:, :rows].rearrange("p (s e) -> p s e", e=cfg.hid))

            def project(src_dram, n_pad, w_t, b_t, out_dram, out_width, odt=F32):
                for g in range(n_pad // PT):
                    xt_t = ppool.tile([128, PT], F32, tag="xt_t")
                    nc.sync.dma_start(xt_t[:], src_dram[:, g * PT:(g + 1) * PT])
                    out_sb = ppool.tile([128, PT // 128, out_width], odt, tag="out_sb")
                    for s in range(PT // 128):
                        ps = psum.tile([128, out_width], F32)
                        nc.tensor.matmul(
                            ps[:], xt_t[:, s * 128:(s + 1) * 128], w_t[:],
                            start=True, stop=True,
                        )
                        nc.vector.tensor_add(out_sb[:, s, :], ps[:], b_t[:])
                    dview = out_dram[g * PT:(g + 1) * PT, :].rearrange(
                        "(s p) e -> p s e", p=128)
                    nc.sync.dma_start(dview, out_sb[:])

            project(xt, cfg.np_pad, w_kv_t, b_kv_t, kv_tab, 2 * cfg.hid, KVDT)
            project(xt_own, cfg.own_pad, w_q_t, b_q_t, q_tab, cfg.hid)

            for ch in range(cfg.n_chunks):
                kv_src = kv_tab[ch * cfg.cr:(ch + 1) * cfg.cr, :]
                for b in range(g_pad // B):
                    c0 = b * (B // 16)
                    kvi = ipool.tile([128, B // 16], I16, tag="kvi")
                    qi = ipool.tile([128, B // 16], I16, tag="qi")
                    sci = ipool.tile([128, B // 16], I16, tag="sci")
                    nc.sync.dma_start(kvi[:], kv_idx[ch, :, c0:c0 + B // 16])
                    nc.sync.dma_start(qi[:], q_idx[ch, :, c0:c0 + B // 16])
                    nc.sync.dma_start(sci[:], sc_idx[ch, :, c0:c0 + B // 16])

                    kv_t = epool.tile([128, BC, 2 * cfg.hid], KVDT, tag="kv_t")
                    q_t = epool.tile([128, BC, cfg.hid], F32, tag="q_t")
                    bi = ch * (g_pad // B) + b
                    nc.gpsimd.dma_gather(
                        kv_t[:], kv_src, kvi[:], B, B, 2 * cfg.hid,
                        queue_num=1 + (2 * bi) % 3)
                    nc.gpsimd.dma_gather(
                        q_t[:], q_tab[:], qi[:], B, B, cfg.hid,
                        queue_num=1 + (2 * bi + 1) % 3)

                    prod = epool.tile([128, BC, cfg.hid], F32, tag="prod")
                    nc.vector.tensor_mul(prod[:], kv_t[:, :, :cfg.hid], q_t[:])

                    sc = epool.tile([128, BC, cfg.heads], F32, tag="sc")
                    nc.vector.reduce_sum(
                        sc[:],
                        prod[:].rearrange("p c (h d) -> p c h d", d=cfg.hdim),
                        axis=mybir.AxisListType.X,
                    )
                    # clip(dot/scale, ±5) == clip(dot, ±5*scale) then /scale
                    lim = 5.0 * float(np.sqrt(cfg.hdim))
                    nc.vector.tensor_scalar_min(sc[:], sc[:], lim)
                    nc.vector.tensor_scalar_max(sc[:], sc[:], -lim)
                    ex = epool.tile([128, BC, cfg.heads], F32, tag="ex")
                    nc.scalar.activation(
                        ex[:], sc[:], mybir.ActivationFunctionType.Exp,
                        scale=float(1.0 / np.sqrt(cfg.hdim)),
                    )

                    msg = epool.tile([128, BC, cfg.hid], F32, tag="msg")
                    nc.vector.tensor_mul(
                        msg[:].rearrange("p c (h d) -> p c h d", d=cfg.hdim),
                        kv_t[:, :, cfg.hid:].rearrange(
                            "p c (h d) -> p c h d", d=cfg.hdim),
                        ex[:].unsqueeze(-1).broadcast_to(
                            [128, BC, cfg.heads, cfg.hdim]),
                    )
                    nc.gpsimd.dma_scatter_add(
                        wv[:], msg[:], sci[:], B, B, cfg.hid)
    nc.finalize()
    return nc


def _wrap16(a, g_pad):
    """[n] -> [128, g_pad//16] int16: idx i at [i%16 (+16k replicas), i//16]."""
    w = a.reshape(g_pad // 16, 16).T.astype(np.int16)  # [16, W]
    return np.tile(w, (8, 1))


def _schedule_batches(dst_local, batch):
    """Assign edges to batches of size `batch` so that no dst row appears
    twice within one batch (dma_scatter_add RMW races on duplicate rows
    within a single call). Returns (n_batches, edge order as an index
    array grouped by batch, per-batch counts)."""
    cnt = len(dst_local)
    if cnt == 0:
        return 1, np.empty(0, np.int64), np.zeros(1, np.int64)
    order = np.argsort(dst_local, kind="stable")
    uniq, starts, degs = np.unique(
        dst_local[order], return_index=True, return_counts=True)
    nb = max(-(-cnt // batch), int(degs.max()))
    big_first = np.argsort(-degs, kind="stable")
    while True:
        fills = np.zeros(nb, np.int64)
        bin_of = np.empty(cnt, np.int64)
        ok = True
        for gi in big_first:
            d = degs[gi]
            cand = np.argsort(fills, kind="stable")[:d]
            if fills[cand[-1]] >= batch:
                ok = False
                break
            fills[cand] += 1
            s = starts[gi]
            bin_of[order[s:s + d]] = cand
        if ok:
            break
        nb += 1
    batch_order = np.argsort(bin_of, kind="stable")
    counts = np.bincount(bin_of, minlength=nb)
    return nb, batch_order, counts


def prepare_inputs(cfg, x, src, dst, Wq, bq, Wk, bk, Wv, bv):
    x = np.asarray(x, np.float32)
    src = np.asarray(src, np.int64)
    dst = np.asarray(dst, np.int64)

    xt = np.zeros((cfg.in_dim, cfg.np_pad), np.float32)
    xt[:, :cfg.n_nodes] = x.T
    w_kv = np.concatenate([np.asarray(Wk, np.float32),
                           np.asarray(Wv, np.float32)], axis=1)
    b_kv = np.tile(np.concatenate([np.asarray(bk, np.float32),
                                   np.asarray(bv, np.float32)])[None, :], (128, 1))
    w_q = np.asarray(Wq, np.float32)
    b_q = np.tile(np.asarray(bq, np.float32)[None, :], (128, 1))

    core_of = dst // cfg.own
    chunk_of = src // cfg.cr

    # per-(core, chunk) edge lists, scheduled into duplicate-free batches
    groups = {}
    nb_max = 1
    for c in range(cfg.n_cores):
        in_c = np.nonzero(core_of == c)[0]
        ch_c = chunk_of[in_c]
        for ch in range(cfg.n_chunks):
            e = in_c[ch_c == ch]
            nb, border, counts = _schedule_batches(
                (dst[e] - c * cfg.own), cfg.batch)
            groups[(c, ch)] = (e[border] if len(e) else e, counts)
            nb_max = max(nb_max, nb)
    g_pad = nb_max * cfg.batch

    in_maps = []
    for c in range(cfg.n_cores):
        kvi = np.zeros((cfg.n_chunks, 128, g_pad // 16), np.int16)
        qi = np.zeros((cfg.n_chunks, 128, g_pad // 16), np.int16)
        sci = np.zeros((cfg.n_chunks, 128, g_pad // 16), np.int16)
        for ch in range(cfg.n_chunks):
            e, counts = groups[(c, ch)]
            kv_l = np.zeros(g_pad, np.int64)
            q_l = np.zeros(g_pad, np.int64)
            sc_l = np.full(g_pad, cfg.dummy_row, np.int64)
            pos = 0
            off = 0
            for b, cnt in enumerate(counts):
                eb = e[pos:pos + cnt]
                kv_l[off:off + cnt] = src[eb] - ch * cfg.cr
                q_l[off:off + cnt] = dst[eb] - c * cfg.own
                sc_l[off:off + cnt] = dst[eb] - c * cfg.own
                pos += cnt
                off += cfg.batch
            kvi[ch] = _wrap16(kv_l, g_pad)
            qi[ch] = _wrap16(q_l, g_pad)
            sci[ch] = _wrap16(sc_l, g_pad)

        xt_own = np.zeros((cfg.in_dim, cfg.own_pad), np.float32)
        xt_own[:, :cfg.own] = x[c * cfg.own:(c + 1) * cfg.own].T
        in_maps.append({
            "xt": xt, "xt_own": xt_own,
            "w_kv": w_kv, "w_q": w_q, "b_kv": b_kv, "b_q": b_q,
            "kv_idx": kvi, "q_idx": qi, "sc_idx": sci,
        })
    return in_maps, g_pad


def kernel(x, src, dst, Wq, bq, Wk, bk, Wv, bv):
    cfg = Cfg()
    in_maps, g_pad = prepare_inputs(cfg, x, src, dst, Wq, bq, Wk, bk, Wv, bv)
    nc = build_program(cfg, g_pad)
    res = run_bass_kernel_spmd(nc, in_maps, list(range(cfg.n_cores)))
    out = np.concatenate(
        [res.results[c]["wv"][:cfg.own] for c in range(cfg.n_cores)], axis=0)
    return out.reshape(cfg.n_nodes, cfg.heads, cfg.hdim)



# revision 7
# speedup vs baseline: 3.1282x; 2.1075x over previous
"""Multi-head graph attention message passing on 8 Trainium2 cores.

Strategy (dst-sharded, one SWDGE gather per edge):
  - Nodes sharded by dst across 8 cores (12500 each).
  - Per core, edges split into 4 src-windows of 25600 nodes so gather
    indices fit int16. Per window, own dsts are sorted by window-degree
    and grouped into blocks of 128; a dst's rank%128 picks its SBUF
    partition, so:
      * Q[dst] is a per-partition broadcast from an SBUF-resident Q table
        (no per-edge Q gather), and
      * segment_sum is a DVE reduce over the free dim (no scatter-add).
  - The only per-edge SWDGE op is the K|V row gather (bf16, 512B rows),
    spread round-robin over 4 SWDGE queues so Q7 descriptor generation
    parallelizes across core pairs.
  - K|V tables bf16; scores/wv accumulate fp32; outputs bf16, summed and
    unpermuted on the host across windows.
"""

import numpy as np
import ml_dtypes

import concourse.bacc as bacc
import concourse.mybir as mybir
import concourse.tile as tile
from concourse.bass_utils import run_bass_kernel_spmd

F32 = mybir.dt.float32
BF16 = mybir.dt.bfloat16
I16 = mybir.dt.int16


class Cfg:
    n_nodes = 100000
    n_edges = 1600000
    in_dim = 128
    heads = 8
    hdim = 16
    hid = 128
    n_cores = 8
    own = 12500
    own_pad = 12544          # 98 blocks of 128
    n_blocks = 98
    nw = 4                   # src windows
    win = 25600              # nodes per window
    rowstride = 25601        # +1 zero pad row per window
    pad_idx = 25600          # window-relative pad row index
    seg_cols = 32            # target cols per gathered segment tile
    gat_cols = 16            # max cols per dma_gather call (2048 idxs)
    op_cols = 32             # max cols per DVE op group
    proj_tile = 512

    def __init__(self, **kw):
        for k, v in kw.items():
            setattr(self, k, v)
        self.kv_rows = self.nw * self.rowstride          # 102404
        self.np_pad = -(-self.kv_rows // self.proj_tile) * self.proj_tile
        self.qn_pad = 12800   # own_pad padded to proj_tile multiple


def make_plan(cfg, widths):
    """widths: [nw][n_blocks] static col-widths (max over cores).
    Returns per-window segment/opgroup/gather structure + idx layout."""
    plan = []
    col = 0
    for w in range(cfg.nw):
        wl = widths[w]
        zb = cfg.n_blocks
        for b in range(cfg.n_blocks):
            if wl[b] == 0:
                zb = b
                break
        segs = []
        b = 0
        while b < zb:
            b0, cols = b, 0
            while b < zb and (cols == 0 or cols + wl[b] <= cfg.seg_cols):
                cols += wl[b]
                b += 1
            # op groups: runs of equal W, each <= op_cols, k <= 8
            ops = []
            oc = 0
            bb = b0
            while bb < b:
                W = wl[bb]
                k = 1
                while (bb + k < b and wl[bb + k] == W and k < 8
                       and (k + 1) * W <= cfg.op_cols):
                    k += 1
                ops.append((bb, k, W, oc))
                oc += k * W
                bb += k
            # gather calls: split cols into chunks <= gat_cols
            gats = []
            gc = 0
            while gc < cols:
                n = min(cfg.gat_cols, cols - gc)
                gats.append((gc, n))
                gc += n
            segs.append(dict(w=w, b0=b0, nb=b - b0, cols=cols,
                             col0=col, ops=ops, gats=gats))
            col += cols
        plan.append(dict(w=w, zb=zb, segs=segs))
    return plan, col  # col == total cols


def build_program(cfg, widths):
    plan, tot_cols = make_plan(cfg, widths)
    tot_idx = 128 * tot_cols
    segc = max(s["cols"] for pw in plan for s in pw["segs"])
    opc = max(k * W for pw in plan for s in pw["segs"]
              for (_, k, W, _) in s["ops"])
    opk = max(k for pw in plan for s in pw["segs"]
              for (_, k, W, _) in s["ops"])

    nc = bacc.Bacc("TRN2", target_bir_lowering=False, debug=False,
                   num_swdge_queues=4)

    xt = nc.dram_tensor("xt", [cfg.in_dim, cfg.np_pad], BF16,
                        kind="ExternalInput")
    xt_own = nc.dram_tensor("xt_own", [cfg.nw, cfg.in_dim, cfg.qn_pad],
                            BF16, kind="ExternalInput")
    w_kv = nc.dram_tensor("w_kv", [cfg.in_dim, 2 * cfg.hid], BF16,
                          kind="ExternalInput")
    b_kv = nc.dram_tensor("b_kv", [128, 2 * cfg.hid], BF16,
                          kind="ExternalInput")
    w_q = nc.dram_tensor("w_q", [cfg.in_dim, cfg.hid], BF16,
                         kind="ExternalInput")
    b_q = nc.dram_tensor("b_q", [128, cfg.hid], BF16, kind="ExternalInput")
    idx = nc.dram_tensor("idx", [128, tot_idx // 16], I16,
                         kind="ExternalInput")
    wv = nc.dram_tensor("wv", [cfg.nw * cfg.own_pad, cfg.hid], BF16,
                        kind="ExternalOutput")

    kv_tab = nc.dram_tensor("kv_tab", [cfg.np_pad, 2 * cfg.hid], BF16)

    PT = cfg.proj_tile
    lim = 5.0 * float(np.sqrt(cfg.hdim))
    gq = [0]  # gather queue round robin

    with tile.TileContext(nc) as tc:
        with (
            tc.tile_pool(name="const", bufs=1) as cpool,
            tc.tile_pool(name="proj", bufs=3) as ppool,
            tc.tile_pool(name="psum", bufs=2, space="PSUM") as psum,
            tc.tile_pool(name="qsum", bufs=2, space="PSUM") as qsum,
            tc.tile_pool(name="qtab", bufs=1) as qpool,
            tc.tile_pool(name="seg", bufs=2) as spool,
            tc.tile_pool(name="edge", bufs=2) as epool,
        ):
            w_kv_t = cpool.tile([cfg.in_dim, 2 * cfg.hid], BF16)
            b_kv_t = cpool.tile([128, 2 * cfg.hid], BF16)
            w_q_t = cpool.tile([cfg.in_dim, cfg.hid], BF16)
            b_q_t = cpool.tile([128, cfg.hid], BF16)
            nc.sync.dma_start(w_kv_t[:], w_kv[:])
            nc.sync.dma_start(b_kv_t[:], b_kv[:])
            nc.sync.dma_start(w_q_t[:], w_q[:])
            nc.sync.dma_start(b_q_t[:], b_q[:])
            idx_t = cpool.tile([128, tot_idx // 16], I16)
            nc.sync.dma_start(idx_t[:], idx[:])
            zt = cpool.tile([128, cfg.hid], BF16)
            nc.vector.memset(zt[:], 0.0)

            # K|V projection of all windows' nodes -> kv_tab (bf16)
            for g in range(cfg.np_pad // PT):
                xt_t = ppool.tile([128, PT], BF16, tag="xt_t")
                nc.sync.dma_start(xt_t[:], xt[:, g * PT:(g + 1) * PT])
                ps = psum.tile([128, PT // 128, 2 * cfg.hid], F32)
                for s in range(PT // 128):
                    nc.tensor.matmul(
                        ps[:, s, :], xt_t[:, s * 128:(s + 1) * 128],
                        w_kv_t[:], start=True, stop=True)
                out_sb = ppool.tile([128, PT // 128, 2 * cfg.hid], BF16,
                                    tag="out_sb")
                nc.vector.tensor_add(
                    out_sb[:], ps[:],
                    b_kv_t[:].unsqueeze(1).broadcast_to(
                        [128, PT // 128, 2 * cfg.hid]))
                nc.sync.dma_start(
                    kv_tab[g * PT:(g + 1) * PT, :].rearrange(
                        "(s p) e -> p s e", p=128),
                    out_sb[:])

            # zero the V half of each window's pad row
            for w in range(cfg.nw):
                r = w * cfg.rowstride + cfg.win
                nc.sync.dma_start(kv_tab[r:r + 1, cfg.hid:], zt[:1, :])

            for w in range(cfg.nw):
                pw = plan[w]
                # Q projection for this window's permuted own nodes
                q_sb = qpool.tile([128, cfg.qn_pad // 128, cfg.hid], BF16,
                                  tag="q_sb")
                for g in range(cfg.qn_pad // PT):
                    xo_t = ppool.tile([128, PT], BF16, tag="xo_t")
                    nc.sync.dma_start(
                        xo_t[:], xt_own[w, :, g * PT:(g + 1) * PT])
                    qs = qsum.tile([128, PT // 128, cfg.hid], F32)
                    for s in range(PT // 128):
                        nc.tensor.matmul(
                            qs[:, s, :], xo_t[:, s * 128:(s + 1) * 128],
                            w_q_t[:], start=True, stop=True)
                    nc.vector.tensor_add(
                        q_sb[:, g * (PT // 128):(g + 1) * (PT // 128), :],
                        qs[:],
                        b_q_t[:].unsqueeze(1).broadcast_to(
                            [128, PT // 128, cfg.hid]))

                kv_win = kv_tab[w * cfg.rowstride:
                                w * cfg.rowstride + cfg.rowstride, :]
                for seg in pw["segs"]:
                    cols = seg["cols"]
                    kv_t = spool.tile([128, segc, 2 * cfg.hid],
                                      BF16, tag="kv_t")
                    for (gc, ncol) in seg["gats"]:
                        n = 128 * ncol
                        o = (seg["col0"] + gc) * 8  # 128/16 per col
                        nc.gpsimd.dma_gather(
                            kv_t[:, gc:gc + ncol, :], kv_win,
                            idx_t[:, o:o + ncol * 8], n, n, 2 * cfg.hid,
                            queue_num=gq[0] % 4, single_packet=False)
                        gq[0] += 1

                    for (b0, k, W, oc) in seg["ops"]:
                        kW = k * W
                        kview = kv_t[:, oc:oc + kW, :cfg.hid]
                        vview = kv_t[:, oc:oc + kW, cfg.hid:]
                        prod = epool.tile([128, opc, cfg.hid],
                                          BF16, tag="prod")
                        nc.vector.tensor_mul(
                            prod[:, :kW, :].rearrange(
                                "p (k u) f -> p k u f", k=k),
                            kview.rearrange("p (k u) f -> p k u f", k=k),
                            q_sb[:, b0:b0 + k, :].unsqueeze(2).broadcast_to(
                                [128, k, W, cfg.hid]))
                        sc = epool.tile([128, opc, cfg.heads],
                                        F32, tag="sc")
                        nc.vector.reduce_sum(
                            sc[:, :kW, :],
                            prod[:, :kW, :].rearrange(
                                "p c (h d) -> p c h d", d=cfg.hdim),
                            axis=mybir.AxisListType.X)
                        nc.vector.tensor_scalar(
                            sc[:, :kW, :], sc[:, :kW, :], lim, -lim,
                            mybir.AluOpType.min, mybir.AluOpType.max)
                        ex = epool.tile([128, opc, cfg.heads],
                                        BF16, tag="ex")
                        nc.scalar.activation(
                            ex[:, :kW, :], sc[:, :kW, :],
                            mybir.ActivationFunctionType.Exp,
                            scale=float(1.0 / np.sqrt(cfg.hdim)))
                        msg = epool.tile([128, opc, cfg.hid],
                                         BF16, tag="msg")
                        nc.vector.tensor_mul(
                            msg[:, :kW, :].rearrange(
                                "p c (h d) -> p c h d", d=cfg.hdim),
                            vview.rearrange(
                                "p c (h d) -> p c h d", d=cfg.hdim),
                            ex[:, :kW, :].unsqueeze(-1).broadcast_to(
                                [128, kW, cfg.heads, cfg.hdim]))
                        wvb = epool.tile([128, opk, cfg.hid], F32, tag="wvb")
                        nc.vector.reduce_sum(
                            wvb[:, :k, :],
                            msg[:, :kW, :].rearrange(
                                "p (k u) f -> p k f u", k=k),
                            axis=mybir.AxisListType.X)
                        wvc = epool.tile([128, opk, cfg.hid], BF16, tag="wvc")
                        nc.scalar.copy(wvc[:, :k, :], wvb[:, :k, :])
                        r0 = w * cfg.own_pad + 128 * b0
                        nc.sync.dma_start(
                            wv[r0:r0 + 128 * k, :].rearrange(
                                "(s p) e -> p s e", p=128),
                            wvc[:, :k, :])
    nc.finalize()
    return nc


def _wrap16(a):
    n = len(a)
    w = a.reshape(n // 16, 16).T.astype(np.int16)
    return np.tile(w, (8, 1))


def _cumcount(sorted_vals):
    n = len(sorted_vals)
    if n == 0:
        return np.empty(0, np.int64)
    flag = np.empty(n, bool)
    flag[0] = True
    flag[1:] = sorted_vals[1:] != sorted_vals[:-1]
    starts = np.flatnonzero(flag)
    reps = np.diff(np.append(starts, n))
    return np.arange(n) - np.repeat(starts, reps)


def prepare_inputs(cfg, x, src, dst, Wq, bq, Wk, bk, Wv, bv):
    bf = ml_dtypes.bfloat16
    x = np.asarray(x, np.float32)
    src = np.asarray(src, np.int64)
    dst = np.asarray(dst, np.int64)

    # kv table node layout: window w node n -> row w*rowstride + (n - w*win)
    xt = np.zeros((cfg.in_dim, cfg.np_pad), bf)
    for w in range(cfg.nw):
        n0 = w * cfg.win
        n1 = min(cfg.n_nodes, n0 + cfg.win)
        xt[:, w * cfg.rowstride:w * cfg.rowstride + (n1 - n0)] = \
            x[n0:n1].T.astype(bf)

    w_kv = np.concatenate([np.asarray(Wk, np.float32),
                           np.asarray(Wv, np.float32)], axis=1).astype(bf)
    b_kv = np.tile(np.concatenate(
        [np.asarray(bk, np.float32), np.asarray(bv, np.float32)])[None, :],
        (128, 1)).astype(bf)
    w_q = np.asarray(Wq, np.float32).astype(bf)
    b_q = np.tile(np.asarray(bq, np.float32)[None, :], (128, 1)).astype(bf)

    core_of = dst // cfg.own
    win_of = src // cfg.win

    percore = []
    for c in range(cfg.n_cores):
        in_c = np.nonzero(core_of == c)[0]
        s_c, d_c = src[in_c], dst[in_c] - c * cfg.own
        w_c = win_of[in_c]
        wins = []
        for w in range(cfg.nw):
            m = w_c == w
            s_w, d_w = s_c[m], d_c[m]
            deg = np.bincount(d_w, minlength=cfg.own_pad)
            order = np.argsort(-deg, kind="stable")
            rank = np.empty(cfg.own_pad, np.int64)
            rank[order] = np.arange(cfg.own_pad)
            wins.append((s_w, d_w, deg, order, rank))
        percore.append(wins)

    # static width table: max over cores of block-leading degree
    widths = []
    for w in range(cfg.nw):
        wl = np.zeros(cfg.n_blocks, np.int64)
        for c in range(cfg.n_cores):
            deg, order = percore[c][w][2], percore[c][w][3]
            wl = np.maximum(wl, deg[order[::128][:cfg.n_blocks]])
        widths.append(wl.tolist())

    plan, tot_cols = make_plan(cfg, widths)
    tot_idx = 128 * tot_cols

    # per-window global col start of each block
    colstart = np.zeros((cfg.nw, cfg.n_blocks), np.int64)
    for w in range(cfg.nw):
        pw = plan[w]
        for seg in pw["segs"]:
            cc = seg["col0"]
            for b in range(seg["b0"], seg["b0"] + seg["nb"]):
                colstart[w][b] = cc
                cc += widths[w][b]

    in_maps = []
    orders = []
    for c in range(cfg.n_cores):
        idx_all = np.full(tot_idx, cfg.pad_idx, np.int64)
        xo = np.zeros((cfg.nw, cfg.in_dim, cfg.qn_pad), bf)
        ords = []
        for w in range(cfg.nw):
            s_w, d_w, deg, order, rank = percore[c][w]
            ords.append(order)
            r = rank[d_w]
            o2 = np.argsort(r, kind="stable")
            rs = r[o2]
            cc = _cumcount(rs)
            p = rs % 128
            b = rs // 128
            col = colstart[w][b] + cc
            pos = col * 128 + p
            idx_all[pos] = s_w[o2] - w * cfg.win
            # permuted own x for Q projection
            valid = order < cfg.own
            xsel = np.zeros((cfg.own_pad, cfg.in_dim), np.float32)
            xsel[valid] = x[c * cfg.own + order[valid]]
            xo[w, :, :cfg.own_pad] = xsel.T.astype(bf)
        in_maps.append({
            "xt": xt, "xt_own": xo, "w_kv": w_kv, "b_kv": b_kv,
            "w_q": w_q, "b_q": b_q, "idx": _wrap16(idx_all),
        })
        orders.append(ords)
    return in_maps, widths, plan, orders


def assemble(cfg, plan, orders, results):
    out = np.zeros((cfg.n_nodes, cfg.hid), np.float32)
    for c in range(cfg.n_cores):
        wv = results[c]["wv"].astype(np.float32)
        for w in range(cfg.nw):
            zb = plan[w]["zb"]
            nrow = 128 * zb
            h = wv[w * cfg.own_pad: w * cfg.own_pad + nrow]
            order = orders[c][w][:nrow]
            valid = order < cfg.own
            out[c * cfg.own + order[valid]] += h[valid]
    return out.reshape(cfg.n_nodes, cfg.heads, cfg.hdim)


def run(inputs, trace=False):
    cfg = Cfg()
    in_maps, widths, plan, orders = prepare_inputs(cfg, **inputs)
    nc = build_program(cfg, widths)
    res = run_bass_kernel_spmd(nc, in_maps, list(range(cfg.n_cores)),
                               trace=trace)
    return assemble(cfg, plan, orders, res.results), res


def kernel(x, src, dst, Wq, bq, Wk, bk, Wv, bv):
    out, _ = run(dict(x=x, src=src, dst=dst, Wq=Wq, bq=bq,
                      Wk=Wk, bk=bk, Wv=Wv, bv=bv))
    return out
